# revision 1
# baseline (speedup 1.0000x reference)
"""Trainium2 Bass kernel for nn_DGNRNetwork (2-layer TransformerConv GNN).

Strategy (8 NeuronCores, SPMD single NEFF):
  - All node tables (x, k/v projections, encoder output, conv outputs) are
    stored in AllGather-row order g(n) = owner(n)*nag + local_pos(n), so
    conv1 and conv2 share one index space and one edge layout.
  - Destination nodes are bucketed by the pair of padded half-degrees
    (edges split at table row 5*nag so each half-table has < 32768 rows for
    int16 dma_gather indices) and dealt round-robin across cores, giving
    identical shapes and balanced work. Each dst owns fixed-width windows
    of slot columns in one SBUF partition per half; segment softmax becomes
    fixed-window free-dim reductions, with the B-half accumulated on top.
  - Per-edge k/v rows are fetched with dma_gather (<=1024 rows per call);
    conv outputs are written back with dma_scatter_add; small node-level
    gathers use one-index-per-partition indirect DMA.
  - conv1 -> conv2 crosses cores with one AllGather of the h1 shards.
"""

import os
import sys

import numpy as np

for _p in ("/opt/trn_rl_repo", "/root/.axon_site/_ro/trn_rl_repo"):
    if os.path.isdir(_p) and _p not in sys.path:
        sys.path.append(_p)

# problem constants
N = 50000
E = 800000
B = 1000
IN_DIM = 64
HID = 32
HEADS = 4
D1 = HID * HEADS  # 128
OUT_DIM = 5
NCORES = 8
P = 128

PAD_LIST = ([0,1,2,3,4,5,6,7,8,9,10,11,12,13,14,15,16,18,20,22,24,28,32,40,48,64]
            if os.environ.get("K_FINE_PAD") else
            [0, 2, 4, 6, 8, 10, 12, 14, 16, 18, 20, 22, 24, 28, 32, 40, 48, 64])
CHUNK_SLOTS = int(os.environ.get("K_CHUNK_SLOTS", "28"))
GATHER_COLS = 8    # max slot columns per dma_gather call (1024 indices)
NODE_CHUNK = 512   # nodes per node-phase chunk


def _round_up(a, m):
    return (a + m - 1) // m * m


# --------------------------------------------------------------------------
# host-side layout
# --------------------------------------------------------------------------

def build_layout(edge_index, global_indices, n_nodes, n_b, n_cores=NCORES):
    src = np.asarray(edge_index[0], dtype=np.int64)
    dst = np.asarray(edge_index[1], dtype=np.int64)
    gi = np.asarray(global_indices, dtype=np.int64)

    deg = np.bincount(dst, minlength=n_nodes).astype(np.int64)

    # ownership: deal active nodes round-robin in degree order (edge balance)
    active = np.nonzero(deg > 0)[0]
    order_by_deg = active[np.argsort(deg[active], kind="stable")]
    owner = np.full(n_nodes, -1, dtype=np.int32)
    for c in range(n_cores):
        owner[order_by_deg[c::n_cores]] = c
    inactive = np.nonzero(deg == 0)[0]
    for c in range(n_cores):
        owner[inactive[c::n_cores]] = c

    local_pos = np.full(n_nodes, -1, dtype=np.int64)
    counts = np.zeros(n_cores, dtype=np.int64)
    for c in range(n_cores):
        mine = np.nonzero(owner == c)[0]
        local_pos[mine] = np.arange(len(mine))
        counts[c] = len(mine)
    nshard = int(counts.max())
    nag = _round_up(nshard + 1, P)
    NAG = n_cores * nag
    HS = 5 * nag
    assert HS < 32768 and NAG - HS < 32768, nag
    assert NAG % NODE_CHUNK == 0

    g_all = owner.astype(np.int64) * nag + local_pos

    # edges sorted by (dst, g(src)) so the A half is contiguous per dst
    gsrc = g_all[src]
    order = np.lexsort((gsrc, dst))
    sgsrc = gsrc[order].astype(np.int64)
    rowptr = np.zeros(n_nodes + 1, dtype=np.int64)
    np.cumsum(deg, out=rowptr[1:])
    degA = np.bincount(dst[gsrc < HS], minlength=n_nodes).astype(np.int64)
    degB = deg - degA

    pad_arr = np.array(PAD_LIST)

    def pad_of(arr):
        return pad_arr[np.searchsorted(pad_arr, arr, side="left")]

    assert int(degA.max()) <= PAD_LIST[-1] and int(degB.max()) <= PAD_LIST[-1]
    kA_all = np.zeros(n_nodes, dtype=np.int64)
    kB_all = np.zeros(n_nodes, dtype=np.int64)
    kA_all[active] = pad_of(degA[active])
    kB_all[active] = pad_of(degB[active])

    # per-core node order: (padA desc, padB desc, id); 128 nodes per column
    core_sorted = []
    for c in range(n_cores):
        mine = active[owner[active] == c]
        o = np.lexsort((mine, -kB_all[mine], -kA_all[mine]))
        core_sorted.append(mine[o])
    ND = max((len(s) + P - 1) // P for s in core_sorted)

    # unified per-column pads (max across cores)
    CA = np.zeros(ND, dtype=np.int64)
    CB = np.zeros(ND, dtype=np.int64)
    for c in range(n_cores):
        s = core_sorted[c]
        for w in range(ND):
            seg = s[w * P : (w + 1) * P]
            if len(seg):
                CA[w] = max(CA[w], int(kA_all[seg].max()))
                CB[w] = max(CB[w], int(kB_all[seg].max()))
    colbaseA = np.zeros(ND + 1, dtype=np.int64)
    np.cumsum(CA, out=colbaseA[1:])
    KA = int(colbaseA[-1])
    colbaseB = np.zeros(ND + 1, dtype=np.int64)
    np.cumsum(CB, out=colbaseB[1:])
    KB = int(colbaseB[-1])

    # runs of equal-width columns for the kernel loops
    def build_runs(CW, other):
        runs = []
        w = 0
        while w < ND:
            D = int(CW[w])
            w2 = w
            while (
                w2 < ND and int(CW[w2]) == D
                and ((other[w2] > 0) == (other[w] > 0))
            ):
                w2 += 1
            if D > 0:
                runs.append((w, w2 - w, D, bool(other[w] > 0)))
            w = w2
        return runs

    runsA = [(w0, cw, D, False) for (w0, cw, D, _) in build_runs(CA, np.zeros(ND))]
    runsB = build_runs(CB, CA)  # second=True iff column also has an A part

    gi_owner = owner[gi]
    bc = max(int(np.bincount(gi_owner, minlength=n_cores).max()), 1)
    BC = _round_up(bc, P)

    def wrap16(flat):
        a = np.zeros((16, len(flat) // 16), np.int16)
        i = np.arange(len(flat))
        a[i % 16, i // 16] = flat.astype(np.int16)
        return np.tile(a, (8, 1))

    cores = []
    for c in range(n_cores):
        idxA = np.zeros(KA * P, dtype=np.int64)
        idxB = np.zeros(KB * P, dtype=np.int64)
        maskA = np.zeros((P, KA), dtype=np.float32)
        maskB = np.zeros((P, KB), dtype=np.float32)
        q_idx = np.zeros((P, ND), dtype=np.int32)
        scat = np.full(ND * P, nshard, dtype=np.int64)
        nd_node = np.full((P, ND), -1, dtype=np.int64)
        nodes = core_sorted[c]
        n = len(nodes)
        j = np.arange(n)
        p = j % P
        w = j // P
        q_idx[p, w] = g_all[nodes]
        scat[w * P + p] = local_pos[nodes]
        nd_node[p, w] = nodes
        for idxf, maskf, dg, off, sub, colbase in (
            (idxA, maskA, degA, None, 0, colbaseA),
            (idxB, maskB, degB, degA, HS, colbaseB),
        ):
            d = dg[nodes]
            tot = int(d.sum())
            if tot == 0:
                continue
            pe = np.repeat(p, d)
            within = np.arange(tot) - np.repeat(np.cumsum(d) - d, d)
            ce = np.repeat(colbase[w], d) + within
            base = rowptr[nodes] if off is None else rowptr[nodes] + off[nodes]
            e0 = np.repeat(base, d) + within
            vals = sgsrc[e0] - sub
            idxf[ce * P + pe] = vals
            maskf[pe, ce] = 1.0

        sel = np.nonzero(gi_owner == c)[0]
        nsel = len(sel)
        x12_idx = np.zeros((P, BC // P), dtype=np.int32)
        x3_idx = np.full((P, BC // P), nshard, dtype=np.int32)
        osc = np.full((P, BC // P), n_b, dtype=np.int32)
        jj = np.arange(nsel)
        gn = gi[sel]
        x12_idx[jj % P, jj // P] = g_all[gn].astype(np.int32)
        x3_idx[jj % P, jj // P] = local_pos[gn].astype(np.int32)
        osc[jj % P, jj // P] = sel.astype(np.int32)

        cores.append(
            dict(idxA16=wrap16(idxA), idxB16=wrap16(idxB),
                 maskA=maskA, maskB=maskB, q_idx=q_idx,
                 scat16=wrap16(scat), nd_node=nd_node,
                 x12_idx=x12_idx, x3_idx=x3_idx, oscat_idx=osc, sel=sel,
                 idxA_flat=idxA, idxB_flat=idxB, scat_flat=scat)
        )

    meta = dict(
        runsA=runsA, runsB=runsB, CA=CA, CB=CB,
        colbaseA=colbaseA, colbaseB=colbaseB, KA=KA, KB=KB, ND=ND,
        nshard=nshard, nag=nag, NAG=NAG, HS=HS, BC=BC,
        owner=owner, local_pos=local_pos, g_all=g_all,
        n_cores=n_cores, n_nodes=n_nodes, n_b=n_b,
    )
    return meta, cores


# --------------------------------------------------------------------------
# bass program
# --------------------------------------------------------------------------

def build_bass(meta, debug_dump=()):
    import concourse.bass as bass
    import concourse.tile as tile
    from concourse import bacc, mybir
    from concourse.masks import make_identity

    f32 = mybir.dt.float32
    i32 = mybir.dt.int32
    i16 = mybir.dt.int16
    AX = mybir.AxisListType
    OP = mybir.AluOpType
    ACT = mybir.ActivationFunctionType

    n_b = meta["n_b"]
    ND, KA, KB = meta["ND"], meta["KA"], meta["KB"]
    nag, NAG, HS, BC = meta["nag"], meta["NAG"], meta["HS"], meta["BC"]
    runsA, runsB = meta["runsA"], meta["runsB"]
    colbaseA, colbaseB = meta["colbaseA"], meta["colbaseB"]
    invs = float(1.0 / np.sqrt(np.float32(HID)))

    nc = bacc.Bacc(None, target_bir_lowering=False)

    def ein(name, shape, dtype=f32):
        return nc.dram_tensor(name, shape, dtype, kind="ExternalInput")

    xg = ein("xg", [IN_DIM + 1, NAG])
    w1p = ein("w1p", [IN_DIM + 1, HID])
    w2p = ein("w2p", [HID + 1, HID])
    wq1p = ein("wq1p", [HID + 1, D1])
    wk1p = ein("wk1p", [HID + 1, D1])
    wv1p = ein("wv1p", [HID + 1, D1])
    wq2 = ein("wq2", [D1, D1])
    wk2 = ein("wk2", [D1, D1])
    wv2 = ein("wv2", [D1, D1])
    bq2r = ein("bq2r", [P, D1])
    bk2r = ein("bk2r", [P, D1])
    bv2r = ein("bv2r", [P, D1])
    ow1 = ein("ow1", [HID, OUT_DIM])
    ow2 = ein("ow2", [D1, OUT_DIM])
    ow3 = ein("ow3", [D1, OUT_DIM])
    obr = ein("obr", [P, OUT_DIM])
    idxA_d = ein("idxA16", [P, KA * 8], i16)
    idxB_d = ein("idxB16", [P, KB * 8], i16) if KB else None
    maskA_d = ein("maskA", [P, KA])
    maskB_d = ein("maskB", [P, KB]) if KB else None
    qidx_d = ein("q_idx", [P, ND], i32)
    scat_d = ein("scat16", [P, ND * 8], i16)
    dmnd_d = ein("dm_nd", [P, ND])
    dmag_d = ein("dm_ag2", [P, NAG // P])
    x12_d = ein("x12_idx", [P, BC // P], i32)
    x3_d = ein("x3_idx", [P, BC // P], i32)
    osc_d = ein("oscat_idx", [P, BC // P], i32)

    outp = nc.dram_tensor("outp", [n_b + P, OUT_DIM], f32, kind="ExternalOutput")

    k1t = nc.dram_tensor("k1t", [NAG, D1], f32)
    v1t = nc.dram_tensor("v1t", [NAG, D1], f32)
    ht = nc.dram_tensor("ht", [NAG, HID], f32)
    h1shard = nc.dram_tensor("h1shard", [nag, D1], f32)
    h1ag = nc.dram_tensor("h1ag", [NAG, D1], f32, addr_space="Shared")
    k2t = nc.dram_tensor("k2t", [NAG, D1], f32)
    v2t = nc.dram_tensor("v2t", [NAG, D1], f32)
    h2shard = nc.dram_tensor("h2shard", [nag, D1], f32)

    with tile.TileContext(nc) as tc:
        with (
            tc.tile_pool(name="const", bufs=1) as cpool,
            tc.tile_pool(name="work", bufs=int(os.environ.get("K_WORK_BUFS", "3"))) as wpool,
            tc.tile_pool(name="slot", bufs=int(os.environ.get("K_SLOT_BUFS", "4"))) as spool,
            tc.tile_pool(name="big", bufs=1) as bpool,
            tc.tile_pool(name="tmp", bufs=2) as tpool,
            tc.tile_pool(name="reg", bufs=1) as rpool,
            tc.tile_pool(name="ps", bufs=int(os.environ.get("K_PS_BUFS","3")), space="PSUM") as pspool,
            tc.tile_pool(name="pst", bufs=int(os.environ.get("K_PST_BUFS","2")), space="PSUM") as pstpool,
        ):
            def load_const(dram, shape, dtype=f32):
                t = cpool.tile(shape, dtype, tag=f"c_{dram.name}")
                nc.sync.dma_start(out=t[:], in_=dram[:, :])
                return t

            w1s = load_const(w1p, [IN_DIM + 1, HID])
            w2s = load_const(w2p, [HID + 1, HID])
            wq1s = load_const(wq1p, [HID + 1, D1])
            wk1s = load_const(wk1p, [HID + 1, D1])
            wv1s = load_const(wv1p, [HID + 1, D1])
            wq2s = load_const(wq2, [D1, D1])
            wk2s = load_const(wk2, [D1, D1])
            wv2s = load_const(wv2, [D1, D1])
            bq2s = load_const(bq2r, [P, D1])
            bk2s = load_const(bk2r, [P, D1])
            bv2s = load_const(bv2r, [P, D1])
            ow1s = load_const(ow1, [HID, OUT_DIM])
            ow2s = load_const(ow2, [D1, OUT_DIM])
            ow3s = load_const(ow3, [D1, OUT_DIM])
            obs = load_const(obr, [P, OUT_DIM])
            idxAs = load_const(idxA_d, [P, KA * 8], i16)
            idxBs = load_const(idxB_d, [P, KB * 8], i16) if KB else None
            maskAs = load_const(maskA_d, [P, KA])
            maskBs = load_const(maskB_d, [P, KB]) if KB else None
            qidxs = load_const(qidx_d, [P, ND], i32)
            scats = load_const(scat_d, [P, ND * 8], i16)
            dmnds = load_const(dmnd_d, [P, ND])
            dmags = load_const(dmag_d, [P, NAG // P])
            x12s = load_const(x12_d, [P, BC // P], i32)
            x3s = load_const(x3_d, [P, BC // P], i32)
            oscs = load_const(osc_d, [P, BC // P], i32)

            ident = cpool.tile([P, P], f32)
            make_identity(nc, ident[:])
            ZROWS = 2048
            zt = cpool.tile([P, ZROWS // P * D1], f32)
            nc.vector.memset(zt[:], 0.0)

            Areg = rpool.tile([P, ND * D1], f32)   # q1 then q2
            Breg = rpool.tile([P, ND * D1], f32)   # conv out / h1_local
            dreg = rpool.tile([P, ND * HEADS], f32)

            # zero both shard tables early so the fills overlap the node phase
            for shard0 in (h1shard, h2shard):
                for r0 in range(0, nag, ZROWS):
                    rows = min(ZROWS, nag - r0)
                    nc.gpsimd.dma_start(
                        out=shard0[r0 : r0 + rows, :],
                        in_=zt[:, : rows // P * D1],
                    )

            # ---------------- conv1 node phase (g-ordered) ----------------
            for c0 in range(0, NAG, NODE_CHUNK):
                xt_t = wpool.tile([IN_DIM + 1, NODE_CHUNK], f32, tag="xt")
                nc.sync.dma_start(out=xt_t[:], in_=xg[:, c0 : c0 + NODE_CHUNK])
                ps1 = pspool.tile([HID, NODE_CHUNK], f32, tag="mm32")
                nc.tensor.matmul(ps1[:], w1s[:], xt_t[:], start=True, stop=True)
                h1e = wpool.tile([HID + 1, NODE_CHUNK], f32, tag="h1e")
                nc.scalar.activation(h1e[0:HID, :], ps1[:], ACT.Relu)
                nc.vector.memset(h1e[HID : HID + 1, :], 1.0)
                ps2 = pspool.tile([HID, NODE_CHUNK], f32, tag="mm32")
                nc.tensor.matmul(ps2[:], w2s[:], h1e[:], start=True, stop=True)
                hTt = wpool.tile([HID + 1, NODE_CHUNK], f32, tag="hT")
                nc.scalar.activation(hTt[0:HID, :], ps2[:], ACT.Relu)
                nc.vector.memset(hTt[HID : HID + 1, :], 1.0)

                for w_s, table in ((wk1s, k1t), (wv1s, v1t)):
                    pk = pspool.tile([P, NODE_CHUNK], f32, tag="mmk")
                    for j in range(NODE_CHUNK // P):
                        nc.tensor.matmul(
                            pk[:, j * P : (j + 1) * P],
                            hTt[:, j * P : (j + 1) * P],
                            w_s[:],
                            start=True, stop=True,
                        )
                    ksb = wpool.tile([P, NODE_CHUNK], f32, tag="ksb")
                    nc.vector.tensor_copy(ksb[:], pk[:])
                    nc.sync.dma_start(
                        out=table[c0 : c0 + NODE_CHUNK, :].rearrange(
                            "(j p) f -> p j f", p=P
                        ),
                        in_=ksb[:].rearrange("p (j f) -> p j f", f=D1),
                    )

                phb = pstpool.tile([P, NODE_CHUNK // P * HID], f32, tag="t")
                for j in range(NODE_CHUNK // P):
                    nc.tensor.transpose(
                        phb[:, j * HID : (j + 1) * HID],
                        hTt[0:HID, j * P : (j + 1) * P],
                        ident[0:HID, 0:HID],
                    )
                hsb = wpool.tile([P, NODE_CHUNK // P * HID], f32, tag="hsb")
                nc.scalar.copy(hsb[:], phb[:])
                nc.sync.dma_start(
                    out=ht[c0 : c0 + NODE_CHUNK, :].rearrange(
                        "(j p) f -> p j f", p=P
                    ),
                    in_=hsb[:].rearrange("p (j f) -> p j f", f=HID),
                )

            # ---------------- q1 (nd order) ----------------
            hq = rpool.tile([P, ND * HID], f32)
            for w in range(ND):
                nc.gpsimd.indirect_dma_start(
                    out=hq[:, w * HID : (w + 1) * HID],
                    out_offset=None,
                    in_=ht[:, :],
                    in_offset=bass.IndirectOffsetOnAxis(
                        ap=qidxs[:, w : w + 1], axis=0
                    ),
                )
            QB = 4
            for w0 in range(0, ND, QB):
                qn = min(QB, ND - w0)
                pt = pstpool.tile([HID, QB * P], f32, tag="t")
                for j in range(qn):
                    nc.tensor.transpose(
                        pt[:, j * P : (j + 1) * P],
                        hq[:, (w0 + j) * HID : (w0 + j + 1) * HID], ident[:]
                    )
                hqT = tpool.tile([HID + 1, QB * P], f32, tag="hqT")
                nc.scalar.copy(hqT[0:HID, : qn * P], pt[:, : qn * P])
                nc.vector.memset(hqT[HID : HID + 1, :], 1.0)
                pq = pstpool.tile([P, QB * D1], f32, tag="t")
                for j in range(qn):
                    nc.tensor.matmul(
                        pq[:, j * D1 : (j + 1) * D1],
                        hqT[:, j * P : (j + 1) * P], wq1s[:],
                        start=True, stop=True,
                    )
                nc.scalar.copy(
                    Areg[:, w0 * D1 : (w0 + qn) * D1], pq[:, : qn * D1]
                )

            # ---------------- edge phase ----------------
            def gather_cols(dst_tile, cc_total, table, idx_tile, gcol0):
                """Fill dst_tile[:, :cc_total*D1] with gathered rows; slot
                columns [gcol0, gcol0+cc_total) of the grid."""
                for p0 in range(0, cc_total, GATHER_COLS):
                    pc = min(GATHER_COLS, cc_total - p0)
                    ni = pc * P
                    i0 = (gcol0 + p0) * P
                    nc.gpsimd.dma_gather(
                        out_ap=dst_tile[:, p0 * D1 : (p0 + pc) * D1].rearrange(
                            "p (s f) -> p s f", f=D1
                        ),
                        in_ap=table,
                        idxs_ap=idx_tile[:, i0 // 16 : (i0 + ni) // 16],
                        num_idxs=ni,
                        num_idxs_reg=ni,
                        elem_size=D1,
                    )

            def edge_phase(ktab, vtab):
                passes = [
                    (runsA, colbaseA, idxAs, maskAs,
                     ktab[0:HS, :], vtab[0:HS, :]),
                    (runsB, colbaseB, idxBs, maskBs,
                     ktab[HS:NAG, :], vtab[HS:NAG, :]),
                ]
                for runs, colbase, idx_t, mask_t, ktb, vtb in passes:
                    for rw0, rcw, D, second in runs:
                        cw_max = max(1, CHUNK_SLOTS // D)
                        for w0 in range(rw0, rw0 + rcw, cw_max):
                            cw = min(cw_max, rw0 + rcw - w0)
                            cc = cw * D
                            gc0 = int(colbase[w0])
                            nd0 = w0
                            big = D > CHUNK_SLOTS
                            pool_s = bpool if big else spool
                            stag = "bigslot" if big else "slot"
                            kt_t = pool_s.tile([P, cc * D1], f32, tag=stag)
                            gather_cols(kt_t, cc, ktb, idx_t, gc0)
                            qb = (
                                Areg[:, nd0 * D1 : (nd0 + cw) * D1]
                                .rearrange("p (w f) -> p w f", f=D1)
                                .unsqueeze(2)
                                .to_broadcast([P, cw, D, D1])
                            )
                            k4 = kt_t[:, : cc * D1].rearrange(
                                "p (w t f) -> p w t f", t=D, f=D1
                            )
                            nc.vector.tensor_tensor(k4, k4, qb, OP.mult)
                            al = spool.tile([P, cc * HEADS], f32, tag="alpha")
                            nc.vector.tensor_reduce(
                                al[:].rearrange("p (s h) -> p s h", h=HEADS),
                                kt_t[:, : cc * D1].rearrange(
                                    "p (s h c) -> p s h c", h=HEADS, c=HID
                                ),
                                axis=AX.X, op=OP.add,
                            )
                            nc.scalar.activation(al[:], al[:], ACT.Exp, scale=invs)
                            mb = (
                                mask_t[:, gc0 : gc0 + cc]
                                .unsqueeze(2)
                                .to_broadcast([P, cc, HEADS])
                            )
                            a3 = al[:].rearrange("p (s h) -> p s h", h=HEADS)
                            nc.vector.tensor_tensor(a3, a3, mb, OP.mult)
                            # denom windows
                            din = al[:].rearrange(
                                "p (w t h) -> p w h t", t=D, h=HEADS
                            )
                            if second:
                                dtmp = tpool.tile([P, cw * HEADS], f32, tag="dtmp")
                                nc.vector.tensor_reduce(
                                    dtmp[:].rearrange("p (w h) -> p w h", h=HEADS),
                                    din, axis=AX.X, op=OP.add,
                                )
                                dsl = dreg[:, nd0 * HEADS : (nd0 + cw) * HEADS]
                                nc.vector.tensor_add(dsl, dsl, dtmp[:])
                            else:
                                nc.vector.tensor_reduce(
                                    dreg[:, nd0 * HEADS : (nd0 + cw) * HEADS]
                                    .rearrange("p (w h) -> p w h", h=HEADS),
                                    din, axis=AX.X, op=OP.add,
                                )
                            vt_t = pool_s.tile([P, cc * D1], f32, tag=stag)
                            gather_cols(vt_t, cc, vtb, idx_t, gc0)
                            eb = (
                                al[:]
                                .rearrange("p (s h) -> p s h", h=HEADS)
                                .unsqueeze(3)
                                .to_broadcast([P, cc, HEADS, HID])
                            )
                            v4 = vt_t[:, : cc * D1].rearrange(
                                "p (s h c) -> p s h c", h=HEADS, c=HID
                            )
                            nc.vector.tensor_tensor(v4, v4, eb, OP.mult)
                            oin = vt_t[:, : cc * D1].rearrange(
                                "p (w t f) -> p w f t", t=D, f=D1
                            )
                            if second:
                                btmp = tpool.tile([P, cw * D1], f32, tag="btmp")
                                nc.vector.tensor_reduce(
                                    btmp[:].rearrange("p (w f) -> p w f", f=D1),
                                    oin, axis=AX.X, op=OP.add,
                                )
                                bsl = Breg[:, nd0 * D1 : (nd0 + cw) * D1]
                                nc.vector.tensor_add(bsl, bsl, btmp[:])
                            else:
                                nc.vector.tensor_reduce(
                                    Breg[:, nd0 * D1 : (nd0 + cw) * D1]
                                    .rearrange("p (w f) -> p w f", f=D1),
                                    oin, axis=AX.X, op=OP.add,
                                )

            def finalize(shard):
                nc.vector.tensor_scalar_add(dreg[:], dreg[:], 1e-16)
                nc.vector.reciprocal(dreg[:], dreg[:])
                rb = (
                    dreg[:]
                    .rearrange("p (n h) -> p n h", h=HEADS)
                    .unsqueeze(3)
                    .to_broadcast([P, ND, HEADS, HID])
                )
                b4 = Breg[:].rearrange("p (n h c) -> p n h c", h=HEADS, c=HID)
                nc.vector.tensor_tensor(b4, b4, rb, OP.mult)
                nc.scalar.activation(Breg[:], Breg[:], ACT.Relu)
                for w0 in range(0, ND, GATHER_COLS):
                    wc = min(GATHER_COLS, ND - w0)
                    ni = wc * P
                    nc.gpsimd.dma_scatter_add(
                        shard[:, :],
                        Breg[:, w0 * D1 : (w0 + wc) * D1].rearrange(
                            "p (s f) -> p s f", f=D1
                        ),
                        scats[:, w0 * 8 : (w0 + wc) * 8],
                        ni, ni, D1,
                    )

            def dump_sbuf(nm, t, width):
                dbg = nc.dram_tensor(f"dbg_{nm}", [P, width], f32,
                                     kind="ExternalOutput")
                nc.sync.dma_start(out=dbg[:, :], in_=t[:, :width])

            PHASE = float(os.environ.get("K_PHASE", "9"))
            if "q1" in debug_dump:
                dump_sbuf("q1", Areg, ND * D1)

            # ---------------- conv1 ----------------
            if PHASE >= 2:
                edge_phase(k1t, v1t)
            if "den1" in debug_dump:
                dump_sbuf("den1", dreg, ND * HEADS)
            if "braw1" in debug_dump:
                dump_sbuf("braw1", Breg, ND * D1)
            if PHASE >= 2:
                finalize(h1shard)

            # ---------------- AllGather ----------------
            if PHASE >= 2.5:
              nc.gpsimd.collective_compute(
                "AllGather",
                mybir.AluOpType.bypass,
                replica_groups=[list(range(meta["n_cores"]))],
                ins=[h1shard[:, :]],
                outs=[h1ag[:, :]],
              )

            # ---------------- conv2 node phase ----------------
            for c0 in (range(0, NAG, NODE_CHUNK) if PHASE >= 3 else ()):
                hsb2 = wpool.tile([P, NODE_CHUNK // P * D1], f32, tag="hsb2")
                nc.sync.dma_start(
                    out=hsb2[:].rearrange("p (j f) -> p j f", f=D1),
                    in_=h1ag[c0 : c0 + NODE_CHUNK, :].rearrange(
                        "(j p) f -> p j f", p=P
                    ),
                )
                pk_k = pspool.tile([P, NODE_CHUNK], f32, tag="mmk")
                pk_v = pspool.tile([P, NODE_CHUNK], f32, tag="mmk")
                ptr = pstpool.tile([P, NODE_CHUNK], f32, tag="t")
                for j in range(NODE_CHUNK // P):
                    nc.tensor.transpose(
                        ptr[:, j * D1 : (j + 1) * D1],
                        hsb2[:, j * D1 : (j + 1) * D1], ident[:]
                    )
                h1T = tpool.tile([P, NODE_CHUNK], f32, tag="h1T")
                nc.scalar.copy(h1T[:], ptr[:])
                for j in range(NODE_CHUNK // P):
                    nc.tensor.matmul(
                        pk_k[:, j * P : (j + 1) * P],
                        h1T[:, j * P : (j + 1) * P], wk2s[:],
                        start=True, stop=True,
                    )
                    nc.tensor.matmul(
                        pk_v[:, j * P : (j + 1) * P],
                        h1T[:, j * P : (j + 1) * P], wv2s[:],
                        start=True, stop=True,
                    )
                for pk, b_s, table in ((pk_k, bk2s, k2t), (pk_v, bv2s, v2t)):
                    ksb = wpool.tile([P, NODE_CHUNK], f32, tag="ksb")
                    for j in range(NODE_CHUNK // P):
                        col = c0 // P + j
                        nc.vector.scalar_tensor_tensor(
                            ksb[:, j * D1 : (j + 1) * D1],
                            pk[:, j * D1 : (j + 1) * D1],
                            dmags[:, col : col + 1],
                            b_s[:],
                            OP.mult, OP.add,
                        )
                    nc.sync.dma_start(
                        out=table[c0 : c0 + NODE_CHUNK, :].rearrange(
                            "(j p) f -> p j f", p=P
                        ),
                        in_=ksb[:].rearrange("p (j f) -> p j f", f=D1),
                    )

            # ---------------- q2 ----------------
            for w0 in (range(0, ND, QB) if PHASE >= 3 else ()):
                qn = min(QB, ND - w0)
                ptr = pstpool.tile([P, QB * D1], f32, tag="t")
                for j in range(qn):
                    nc.tensor.transpose(
                        ptr[:, j * D1 : (j + 1) * D1],
                        Breg[:, (w0 + j) * D1 : (w0 + j + 1) * D1], ident[:]
                    )
                h1T = tpool.tile([P, QB * D1], f32, tag="h1T")
                nc.scalar.copy(h1T[:, : qn * D1], ptr[:, : qn * D1])
                pq = pstpool.tile([P, QB * D1], f32, tag="t")
                for j in range(qn):
                    nc.tensor.matmul(
                        pq[:, j * D1 : (j + 1) * D1],
                        h1T[:, j * D1 : (j + 1) * D1], wq2s[:],
                        start=True, stop=True,
                    )
                for j in range(qn):
                    nc.vector.scalar_tensor_tensor(
                        Areg[:, (w0 + j) * D1 : (w0 + j + 1) * D1],
                        pq[:, j * D1 : (j + 1) * D1],
                        dmnds[:, w0 + j : w0 + j + 1],
                        bq2s[:],
                        OP.mult, OP.add,
                    )

            # ---------------- conv2 ----------------
            if PHASE >= 4:
                edge_phase(k2t, v2t)
                finalize(h2shard)

            # ---------------- head ----------------
            x1g = rpool.tile([P, BC // P * HID], f32)
            x2g = rpool.tile([P, BC // P * D1], f32)
            x3g = rpool.tile([P, BC // P * D1], f32)
            for j in range(BC // P):
                nc.gpsimd.indirect_dma_start(
                    out=x1g[:, j * HID : (j + 1) * HID],
                    out_offset=None, in_=ht[:, :],
                    in_offset=bass.IndirectOffsetOnAxis(
                        ap=x12s[:, j : j + 1], axis=0),
                )
                nc.gpsimd.indirect_dma_start(
                    out=x2g[:, j * D1 : (j + 1) * D1],
                    out_offset=None, in_=h1ag[:, :],
                    in_offset=bass.IndirectOffsetOnAxis(
                        ap=x12s[:, j : j + 1], axis=0),
                )
                nc.gpsimd.indirect_dma_start(
                    out=x3g[:, j * D1 : (j + 1) * D1],
                    out_offset=None, in_=h2shard[:, :],
                    in_offset=bass.IndirectOffsetOnAxis(
                        ap=x3s[:, j : j + 1], axis=0),
                )
            for j in range(BC // P):
                p1 = pstpool.tile([HID, P], f32, tag="t")
                nc.tensor.transpose(
                    p1[:], x1g[:, j * HID : (j + 1) * HID], ident[:]
                )
                x1T = wpool.tile([HID, P], f32, tag="x1T")
                nc.scalar.copy(x1T[:], p1[:])
                p2 = pstpool.tile([P, P], f32, tag="t")
                nc.tensor.transpose(
                    p2[:], x2g[:, j * D1 : (j + 1) * D1], ident[:]
                )
                x2T = wpool.tile([P, P], f32, tag="x2T")
                nc.scalar.copy(x2T[:], p2[:])
                p3 = pstpool.tile([P, P], f32, tag="t")
                nc.tensor.transpose(
                    p3[:], x3g[:, j * D1 : (j + 1) * D1], ident[:]
                )
                x3T = wpool.tile([P, P], f32, tag="x3T")
                nc.scalar.copy(x3T[:], p3[:])
                po = pstpool.tile([P, OUT_DIM], f32, tag="t")
                nc.tensor.matmul(po[:], x1T[:], ow1s[:], start=True, stop=False)
                nc.tensor.matmul(po[:], x2T[:], ow2s[:], start=False, stop=False)
                nc.tensor.matmul(po[:], x3T[:], ow3s[:], start=False, stop=True)
                osb = wpool.tile([P, OUT_DIM], f32, tag="osb")
                nc.vector.scalar_tensor_tensor(
                    osb[:], po[:], 0.0, obs[:], OP.bypass, OP.add
                )
                nc.gpsimd.indirect_dma_start(
                    out=outp[:, :],
                    out_offset=bass.IndirectOffsetOnAxis(
                        ap=oscs[:, j : j + 1], axis=0
                    ),
                    in_=osb[:],
                    in_offset=None,
                )

            dbg_tabs = {
                "k1t": k1t, "v1t": v1t, "ht": ht, "h1shard": h1shard,
                "h1ag": h1ag, "k2t": k2t, "v2t": v2t, "h2shard": h2shard,
            }
            for nm in debug_dump:
                if nm not in dbg_tabs:
                    continue
                tab = dbg_tabs[nm]
                dbg = nc.dram_tensor(
                    f"dbg_{nm}", list(tab.shape), f32, kind="ExternalOutput"
                )
                nc.sync.dma_start(out=dbg[:, :], in_=tab[:, :])

    nc.finalize()
    return nc


# --------------------------------------------------------------------------
# host packing
# --------------------------------------------------------------------------

def pack_inputs(inputs, meta, cores):
    f32 = np.float32
    n_nodes = meta["n_nodes"]
    nag, NAG = meta["nag"], meta["NAG"]
    g_all = meta["g_all"]

    x = np.asarray(inputs["x"], dtype=f32)
    dm = np.asarray(inputs["dm_mask"], dtype=f32).reshape(-1)

    xg = np.zeros((IN_DIM + 1, NAG), f32)
    xg[:IN_DIM, g_all] = x.T
    xg[IN_DIM, :] = 1.0

    def pk(w, b):
        return np.vstack([np.asarray(w, f32), np.asarray(b, f32)[None, :]])

    common = {
        "xg": xg,
        "w1p": pk(inputs["enc_w1"], inputs["enc_b1"]),
        "w2p": pk(inputs["enc_w2"], inputs["enc_b2"]),
        "wq1p": pk(inputs["c1_wq"], inputs["c1_bq"]),
        "wk1p": pk(inputs["c1_wk"], inputs["c1_bk"]),
        "wv1p": pk(inputs["c1_wv"], inputs["c1_bv"]),
        "wq2": np.asarray(inputs["c2_wq"], f32),
        "wk2": np.asarray(inputs["c2_wk"], f32),
        "wv2": np.asarray(inputs["c2_wv"], f32),
        "bq2r": np.tile(np.asarray(inputs["c2_bq"], f32)[None, :], (P, 1)),
        "bk2r": np.tile(np.asarray(inputs["c2_bk"], f32)[None, :], (P, 1)),
        "bv2r": np.tile(np.asarray(inputs["c2_bv"], f32)[None, :], (P, 1)),
        "ow1": np.asarray(inputs["out_w"], f32)[:HID],
        "ow2": np.asarray(inputs["out_w"], f32)[HID : HID + D1],
        "ow3": np.asarray(inputs["out_w"], f32)[HID + D1 :],
        "obr": np.tile(np.asarray(inputs["out_b"], f32)[None, :], (P, 1)),
    }

    dm_ag = np.zeros(NAG, f32)
    dm_ag[g_all] = dm
    common["dm_ag2"] = np.ascontiguousarray(dm_ag.reshape(-1, P).T)

    in_maps = []
    for c, L in enumerate(cores):
        valid = L["nd_node"] >= 0
        dm_nd = np.where(valid, dm[np.where(valid, L["nd_node"], 0)], 0.0)
        m = dict(common)
        m.update(
            idxA16=L["idxA16"], idxB16=L["idxB16"],
            maskA=L["maskA"], maskB=L["maskB"],
            q_idx=L["q_idx"], scat16=L["scat16"],
            dm_nd=dm_nd.astype(f32),
            x12_idx=L["x12_idx"], x3_idx=L["x3_idx"],
            oscat_idx=L["oscat_idx"],
        )
        if meta["KB"] == 0:
            m.pop("idxB16"), m.pop("maskB")
        in_maps.append({k: np.ascontiguousarray(v) for k, v in m.items()})
    return in_maps


_CACHE = {}


def kernel(**inputs):
    from concourse.bass_utils import run_bass_kernel_spmd

    meta, cores = build_layout(
        inputs["edge_index"], inputs["global_indices"], N, B
    )
    nc = build_bass(meta)
    in_maps = pack_inputs(inputs, meta, cores)

    trace = bool(int(os.environ.get("KERNEL_TRACE", "0")))
    res = run_bass_kernel_spmd(
        nc, in_maps, core_ids=list(range(NCORES)), trace=trace,
    )
    if trace and res.exec_time_ns is not None:
        print(f"HW exec time: {res.exec_time_ns} ns")
        _CACHE["exec_time_ns"] = res.exec_time_ns
        _CACHE["res"] = res

    out = np.zeros((B, OUT_DIM), np.float32)
    for c, L in enumerate(cores):
        sel = L["sel"]
        out[sel] = res.results[c]["outp"][sel]
    return out


if __name__ == "__main__":
    import jax

    cpu = jax.devices("cpu")[0]
    sys.path.insert(0, "/root/problem")
    import reference

    with jax.default_device(cpu):
        inputs = {k: np.asarray(v) for k, v in reference.setup_inputs().items()}
        expected = np.asarray(reference.reference(**inputs))
    got = kernel(**inputs)
    err = np.abs(got - expected).max() / (np.abs(expected).max() + 1e-12)
    print("rel err:", err)

